# revision 5
# baseline (speedup 1.0000x reference)
"""Causal self-attention block (B=4, T=2048, D=1024, H=16) on 8 NeuronCores.

Sharding: core c handles batch b = c//2, head-group hg = c%2 (8 heads each).
Each core computes the qkv projection for its heads, causal attention, and a
partial output projection; the host sums the two head-group partials per batch.

On-core layout (all matmul inputs bf16, fp32 accumulation):
  x^T   [1024, 2048]  (host pre-transposed)        -> SBUF [128, 8, 2048]
  Q^T,K^T produced as [512 ch, 2048 t] c-tiles; a c-tile holds a head PAIR
    (head 2i on partitions 0:64, head 2i+1 on 64:128), which lets the two
    K=64 score matmuls run pair-packed on the PE via row tiling (their psum
    chunks sit in different banks — same-bank concurrent row tiles fault).
  S^T per k-tile: psum [128 k, 2*512] (one chunk per head, one bank each);
    exp(0.125*s) on ScalarE straight out of psum into bf16 P^T.
  AV: lhsT = [V_h | 1] (65 cols) so psum row 64 accumulates the softmax
    denominator for free; normalize = approx-reciprocal + ones-matmul
    broadcast (DVE cannot broadcast across partitions).
  y^T stored pair-stacked [128 j, 2048 t] -> feeds out-proj as lhsT directly.

Schedule: V proj first, then QK(ct)/attention(ct-1) interleaved so the PE has
projection work to fill the gaps while ScalarE paces the exp stream; the last
pair's attention interleaves with the output projection.
"""
import sys
sys.path.insert(0, '/opt/trn_rl_repo')
import numpy as np
import ml_dtypes

B, T, DM = 4, 2048, 1024
H_TOT, DH = 16, 64
HPC = 8            # heads per core
CC = HPC * DH      # 512 channels per core
NDT = DM // 128    # 8 d-tiles
NCT = 4            # c-tiles (head pairs) per core
NTB = 4            # 512-wide t-blocks
NTT = 16           # 128-wide t-tiles
NQB = 4            # 512-wide q-blocks
NKT = 16           # 128-wide k-tiles

_CACHE = {}


def _build():
    import concourse.bacc as bacc
    import concourse.mybir as mybir
    import concourse.tile as tile
    from concourse.masks import make_upper_triangular

    bf16 = mybir.dt.bfloat16
    f32 = mybir.dt.float32
    Exp = mybir.ActivationFunctionType.Exp
    mult = mybir.AluOpType.mult
    add = mybir.AluOpType.add

    nc = bacc.Bacc("TRN2", target_bir_lowering=False, debug=False, num_devices=8)

    xT_d = nc.dram_tensor("xT", [DM, T], bf16, kind="ExternalInput")
    wq_d = nc.dram_tensor("wq", [DM, CC], bf16, kind="ExternalInput")
    wk_d = nc.dram_tensor("wk", [DM, CC], bf16, kind="ExternalInput")
    wv_d = nc.dram_tensor("wv", [DM, CC], bf16, kind="ExternalInput")
    wo_d = nc.dram_tensor("wo", [CC, DM], bf16, kind="ExternalInput")
    bpc_d = nc.dram_tensor("bpc", [128, 2, NCT], f32, kind="ExternalInput")  # q/k bias per (partition, section, c-tile)
    brow_d = nc.dram_tensor("brow", [1, CC], bf16, kind="ExternalInput")      # v bias row
    out_d = nc.dram_tensor("out", [T, DM], f32, kind="ExternalOutput")

    with tile.TileContext(nc) as tc:
        with tc.tile_pool(name="const", bufs=1) as cp, \
             tc.tile_pool(name="work", bufs=4) as wp, \
             tc.tile_pool(name="small", bufs=2) as sp, \
             tc.tile_pool(name="ost", bufs=3) as op_, \
             tc.tile_pool(name="ps_mm", bufs=2, space="PSUM") as ps_mm, \
             tc.tile_pool(name="ps_y", bufs=4, space="PSUM") as ps_y:

            xT = cp.tile([128, NDT, T], bf16, tag="xT")
            wq = cp.tile([128, NDT, CC], bf16, tag="wq")
            wk = cp.tile([128, NDT, CC], bf16, tag="wk")
            wv = cp.tile([128, NDT, CC], bf16, tag="wv")
            wo = cp.tile([128, NCT, DM], bf16, tag="wo")
            bpc = cp.tile([128, 2, NCT], f32, tag="bpc")
            brow = cp.tile([1, CC], bf16, tag="brow")
            ones = cp.tile([1, 512], bf16, tag="ones")
            qt = cp.tile([128, NCT, T], bf16, tag="qt")
            kt = cp.tile([128, NCT, T], bf16, tag="kt")
            vp = cp.tile([128, NKT, HPC, 128], bf16, tag="vp")
            yt = cp.tile([128, NCT, T], bf16, tag="yt")
            bvb = cp.tile([128, HPC, DH], bf16, tag="bvb")

            # gpsimd setup first so it doesn't queue behind DMA issue
            nc.gpsimd.memset(ones[:], 1.0)
            nc.gpsimd.memset(vp[:, :, :, DH:128], 1.0)

            # inbound DMAs spread across two queues (sync + gpsimd issuers)
            xT_src = xT_d.ap().rearrange("(dt p) t -> p dt t", p=128)
            for dt in range(NDT):
                eng = nc.sync if dt % 2 == 0 else nc.gpsimd
                eng.dma_start(xT[:, dt], xT_src[:, dt])
            nc.gpsimd.dma_start(wv[:], wv_d.ap().rearrange("(dt p) c -> p dt c", p=128))
            nc.sync.dma_start(brow[:], brow_d.ap())
            nc.sync.dma_start(wq[:], wq_d.ap().rearrange("(dt p) c -> p dt c", p=128))
            nc.gpsimd.dma_start(wk[:], wk_d.ap().rearrange("(dt p) c -> p dt c", p=128))
            nc.sync.dma_start(bpc[:], bpc_d.ap())
            nc.gpsimd.dma_start(wo[:], wo_d.ap().rearrange("(jt p) c -> p jt c", p=128))

            # v-bias broadcast to all partitions (one-time)
            bv_ps = ps_mm.tile([128, 1024], f32, tag="mm")
            nc.tensor.matmul(bv_ps[:, 0:CC], ones[0:1, 0:128], brow[:], start=True, stop=True)
            nc.vector.tensor_copy(bvb[:], bv_ps[:, 0:CC].rearrange("p (h d) -> p h d", d=DH))

            # ---- V projection: v[t, c] natural layout, packed as [V_h | 1] per head
            for tt in range(NTT):
                v_ps = ps_mm.tile([128, 1024], f32, tag="mm")
                for dt in range(NDT):
                    nc.tensor.matmul(v_ps[:, 0:CC], xT[:, dt, tt * 128:(tt + 1) * 128],
                                     wv[:, dt], start=(dt == 0), stop=(dt == NDT - 1))
                with nc.allow_low_precision("bf16 v with bias"):
                    nc.vector.tensor_tensor(vp[:, tt, :, 0:DH],
                                            v_ps[:, 0:CC].rearrange("p (h d) -> p h d", d=DH),
                                            bvb[:], add)

            def qk_group(ct, w_sb, dst, sec, tb):
                p_ps = ps_mm.tile([128, 1024], f32, tag="mm")
                for dt in range(NDT):
                    nc.tensor.matmul(p_ps[:, 0:512], w_sb[:, dt, ct * 128:(ct + 1) * 128],
                                     xT[:, dt, tb * 512:(tb + 1) * 512],
                                     start=(dt == 0), stop=(dt == NDT - 1))
                with nc.allow_low_precision("bf16 q/k with bias"):
                    nc.vector.tensor_scalar_add(dst[:, ct, tb * 512:(tb + 1) * 512],
                                                p_ps[:, 0:512], bpc[:, sec, ct:ct + 1])

            def qk_groups(ct):
                for w_sb, dst, sec in ((wq, qt, 0), (wk, kt, 1)):
                    for tb in range(NTB):
                        yield lambda w_sb=w_sb, dst=dst, sec=sec, tb=tb: qk_group(ct, w_sb, dst, sec, tb)

            def outproj_tt(tt):
                for ch in range(2):
                    o_ps = ps_mm.tile([128, 1024], f32, tag="mm")
                    for p4 in range(NCT):
                        nc.tensor.matmul(o_ps[:, 0:512], yt[:, p4, tt * 128:(tt + 1) * 128],
                                         wo[:, p4, ch * 512:(ch + 1) * 512],
                                         start=(p4 == 0), stop=(p4 == NCT - 1))
                    o_sb = op_.tile([128, 512], f32, tag="osb")
                    nc.scalar.copy(o_sb[:], o_ps[:, 0:512])
                    nc.sync.dma_start(out_d.ap()[tt * 128:(tt + 1) * 128, ch * 512:(ch + 1) * 512], o_sb[:])

            def attention_block(ct, qb):
                """S -> exp -> AV software-pipelined one k-tile deep."""
                nkt = 4 * (qb + 1)
                yA = ps_y.tile([128, 512], f32, tag="y")
                yB = ps_y.tile([128, 512], f32, tag="y")
                ys = (yA, yB)
                pend = None
                for ki in range(nkt):
                    j = ki - 4 * qb
                    n0 = 128 * j if j >= 0 else 0
                    s_ps = ps_mm.tile([128, 1024], f32, tag="mm")
                    ks = slice(ki * 128, (ki + 1) * 128)
                    qs = slice(qb * 512 + n0, (qb + 1) * 512)
                    nc.tensor.matmul(s_ps[:, n0:512], kt[0:64, ct, ks], qt[0:64, ct, qs],
                                     start=True, stop=True)
                    nc.tensor.matmul(s_ps[:, 512 + n0:1024], kt[64:128, ct, ks], qt[64:128, ct, qs],
                                     start=True, stop=True)
                    pt = wp.tile([128, 2, 512], bf16, tag="pt")
                    s_v = s_ps[:].rearrange("p (c q) -> p c q", q=512)
                    nc.scalar.activation(pt[:, :, n0:512], s_v[:, :, n0:512], Exp, scale=0.125)
                    if j >= 0:
                        nc.gpsimd.affine_select(pt[:, :, n0:n0 + 128], pt[:, :, n0:n0 + 128],
                                                pattern=[[0, 2], [1, 128]],
                                                compare_op=mybir.AluOpType.is_ge,
                                                fill=0.0, base=0, channel_multiplier=-1)
                    if pend is not None:
                        pki, ppt, pn0 = pend
                        for h01 in range(2):
                            nc.tensor.matmul(ys[h01][:, pn0:512], vp[:, pki, 2 * ct + h01, :],
                                             ppt[:, h01, pn0:512],
                                             start=(pki == 0), stop=(pki == nkt - 1))
                    pend = (ki, pt, n0)
                pki, ppt, pn0 = pend
                for h01 in range(2):
                    nc.tensor.matmul(ys[h01][:, pn0:512], vp[:, pki, 2 * ct + h01, :],
                                     ppt[:, h01, pn0:512],
                                     start=(pki == 0), stop=(pki == nkt - 1))
                for h01 in range(2):
                    y_ps = ys[h01]
                    rcp = sp.tile([64, 512], bf16, tag="rcp")
                    with nc.allow_low_precision("bf16 softmax denom recip"):
                        nc.vector.reciprocal(rcp[:], y_ps[64:128, :])
                    with nc.allow_low_precision("bf16 normalized y"):
                        nc.vector.tensor_tensor(yt[64 * h01:64 * (h01 + 1), ct, qb * 512:(qb + 1) * 512],
                                                y_ps[0:64, :], rcp[:], mult)

            # ---- interleaved schedule
            for g in qk_groups(0):
                g()
            for ct in range(NCT):
                filler = list(qk_groups(ct + 1)) if ct + 1 < NCT else None
                for qb in range(NQB):
                    attention_block(ct, qb)
                    if filler is not None:
                        for g in filler[2 * qb:2 * qb + 2]:
                            g()
                    else:
                        for tt in range(4 * qb, 4 * qb + 4):
                            outproj_tt(tt)

    nc.compile()
    return nc


def _get_nc():
    if 'nc' not in _CACHE:
        _CACHE['nc'] = _build()
    return _CACHE['nc']


def _prep_in_maps(x, Wqkv, bqkv, Wout):
    bf = ml_dtypes.bfloat16
    in_maps = []
    per_hg = {}
    for hg in range(2):
        c0 = CC * hg
        wq = np.ascontiguousarray(Wqkv[:, c0:c0 + CC]).astype(bf)
        wk = np.ascontiguousarray(Wqkv[:, DM + c0:DM + c0 + CC]).astype(bf)
        wv = np.ascontiguousarray(Wqkv[:, 2 * DM + c0:2 * DM + c0 + CC]).astype(bf)
        wo = np.ascontiguousarray(Wout[c0:c0 + CC, :]).astype(bf)
        bq = bqkv[c0:c0 + CC]
        bk = bqkv[DM + c0:DM + c0 + CC]
        bv = bqkv[2 * DM + c0:2 * DM + c0 + CC]
        # [128, 2, NCT]: element [p, s, ct] = bias_s[ct*128 + p]
        bpc = np.stack([bq.reshape(NCT, 128).T, bk.reshape(NCT, 128).T], axis=1).astype(np.float32)
        brow = bv.reshape(1, CC).astype(bf)
        per_hg[hg] = dict(wq=wq, wk=wk, wv=wv, wo=wo, bpc=np.ascontiguousarray(bpc), brow=brow)
    for c in range(8):
        b, hg = c // 2, c % 2
        xT = np.ascontiguousarray(x[b].T).astype(bf)
        in_maps.append({"xT": xT, **per_hg[hg]})
    return in_maps


def kernel(x, Wqkv, bqkv, Wout, bout):
    from concourse.bass_utils import run_bass_kernel_spmd
    nc = _get_nc()
    x = np.asarray(x, dtype=np.float32)
    Wqkv = np.asarray(Wqkv, dtype=np.float32)
    bqkv = np.asarray(bqkv, dtype=np.float32)
    Wout = np.asarray(Wout, dtype=np.float32)
    bout = np.asarray(bout, dtype=np.float32)
    in_maps = _prep_in_maps(x, Wqkv, bqkv, Wout)
    res = run_bass_kernel_spmd(nc, in_maps, list(range(8))).results
    out = np.empty((B, T, DM), dtype=np.float32)
    for b in range(B):
        out[b] = res[2 * b]["out"] + res[2 * b + 1]["out"]
    out += bout[None, None, :]
    return out


# revision 6
# speedup vs baseline: 1.1674x; 1.1674x over previous
"""Causal self-attention block (B=4, T=2048, D=1024, H=16) on 8 NeuronCores.

Sharding: core c handles batch b = c//2, head-group hg = c%2 (8 heads each).
Each core computes the qkv projection for its heads, causal attention, and a
partial output projection; the host sums the two head-group partials per batch.

On-core layout (all matmul inputs bf16, fp32 accumulation):
  x^T   [1024, 2048]  (host pre-transposed)        -> SBUF [128, 8, 2048]
  Q^T,K^T produced as [512 ch, 2048 t] c-tiles; a c-tile holds a head PAIR
    (head 2i on partitions 0:64, head 2i+1 on 64:128), which lets the two
    K=64 score matmuls run pair-packed on the PE via row tiling (their psum
    chunks sit in different banks — same-bank concurrent row tiles fault).
  S^T per k-tile: psum [128 k, 2*512] (one chunk per head, one bank each);
    exp(0.125*s) on ScalarE straight out of psum into bf16 P^T.
  AV: lhsT = [V_h | 1] (65 cols) so psum row 64 accumulates the softmax
    denominator for free; normalize = approx-reciprocal + ones-matmul
    broadcast (DVE cannot broadcast across partitions).
  y^T stored pair-stacked [128 j, 2048 t] -> feeds out-proj as lhsT directly.

Schedule: V proj first, then QK(ct)/attention(ct-1) interleaved so the PE has
projection work to fill the gaps while ScalarE paces the exp stream; the last
pair's attention interleaves with the output projection.
"""
import sys
sys.path.insert(0, '/opt/trn_rl_repo')
import numpy as np
import ml_dtypes

B, T, DM = 4, 2048, 1024
H_TOT, DH = 16, 64
HPC = 8            # heads per core
CC = HPC * DH      # 512 channels per core
NDT = DM // 128    # 8 d-tiles
NCT = 4            # c-tiles (head pairs) per core
NTB = 4            # 512-wide t-blocks
NTT = 16           # 128-wide t-tiles
NQB = 4            # 512-wide q-blocks
NKT = 16           # 128-wide k-tiles

_CACHE = {}


def _build():
    import concourse.bacc as bacc
    import concourse.mybir as mybir
    import concourse.tile as tile
    from concourse.masks import make_upper_triangular

    bf16 = mybir.dt.bfloat16
    f32 = mybir.dt.float32
    Exp = mybir.ActivationFunctionType.Exp
    Ln = mybir.ActivationFunctionType.Ln
    mult = mybir.AluOpType.mult
    add = mybir.AluOpType.add

    nc = bacc.Bacc("TRN2", target_bir_lowering=False, debug=False, num_devices=8)

    xT_d = nc.dram_tensor("xT", [DM, T], bf16, kind="ExternalInput")
    wq_d = nc.dram_tensor("wq", [DM, CC], bf16, kind="ExternalInput")
    wk_d = nc.dram_tensor("wk", [DM, CC], bf16, kind="ExternalInput")
    wv_d = nc.dram_tensor("wv", [DM, CC], bf16, kind="ExternalInput")
    wo_d = nc.dram_tensor("wo", [CC, DM], bf16, kind="ExternalInput")
    bpc_d = nc.dram_tensor("bpc", [128, 2, NCT], f32, kind="ExternalInput")  # q/k bias per (partition, section, c-tile)
    brow_d = nc.dram_tensor("brow", [1, CC], bf16, kind="ExternalInput")      # v bias row
    out_d = nc.dram_tensor("out", [T, DM], f32, kind="ExternalOutput")

    with tile.TileContext(nc) as tc:
        with tc.tile_pool(name="const", bufs=1) as cp, \
             tc.tile_pool(name="work", bufs=4) as wp, \
             tc.tile_pool(name="small", bufs=2) as sp, \
             tc.tile_pool(name="ost", bufs=3) as op_, \
             tc.tile_pool(name="ps_mm", bufs=2, space="PSUM") as ps_mm, \
             tc.tile_pool(name="ps_y", bufs=3, space="PSUM") as ps_y, \
             tc.tile_pool(name="ps_qk", bufs=1, space="PSUM") as ps_qk:

            xT = cp.tile([128, NDT, T], bf16, tag="xT")
            wq = cp.tile([128, NDT, CC], bf16, tag="wq")
            wk = cp.tile([128, NDT, CC], bf16, tag="wk")
            wv = cp.tile([128, NDT, CC], bf16, tag="wv")
            wo = cp.tile([128, NCT, DM], bf16, tag="wo")
            bpc = cp.tile([128, 2, NCT], f32, tag="bpc")
            brow = cp.tile([1, CC], bf16, tag="brow")
            ones = cp.tile([1, 512], bf16, tag="ones")
            qt = cp.tile([128, NCT, T], bf16, tag="qt")
            kt = cp.tile([128, NCT, T], bf16, tag="kt")
            vp = cp.tile([128, NKT, HPC, 128], bf16, tag="vp")
            yt = cp.tile([128, NCT, T], bf16, tag="yt")
            bvb = cp.tile([128, HPC, DH], bf16, tag="bvb")

            # gpsimd setup first so it doesn't queue behind DMA issue
            nc.gpsimd.memset(ones[:], 1.0)
            nc.gpsimd.memset(vp[:, :, :, DH:128], 1.0)

            # inbound DMAs spread across two queues (sync + gpsimd issuers)
            xT_src = xT_d.ap().rearrange("(dt p) t -> p dt t", p=128)
            for dt in range(NDT):
                eng = nc.sync if dt % 2 == 0 else nc.gpsimd
                eng.dma_start(xT[:, dt], xT_src[:, dt])
            nc.gpsimd.dma_start(wv[:], wv_d.ap().rearrange("(dt p) c -> p dt c", p=128))
            nc.sync.dma_start(brow[:], brow_d.ap())
            nc.sync.dma_start(wq[:], wq_d.ap().rearrange("(dt p) c -> p dt c", p=128))
            nc.gpsimd.dma_start(wk[:], wk_d.ap().rearrange("(dt p) c -> p dt c", p=128))
            nc.sync.dma_start(bpc[:], bpc_d.ap())
            nc.gpsimd.dma_start(wo[:], wo_d.ap().rearrange("(jt p) c -> p jt c", p=128))

            # v-bias broadcast to all partitions (one-time)
            bv_ps = ps_mm.tile([128, 1024], f32, tag="mm")
            nc.tensor.matmul(bv_ps[:, 0:CC], ones[0:1, 0:128], brow[:], start=True, stop=True)
            nc.vector.tensor_copy(bvb[:], bv_ps[:, 0:CC].rearrange("p (h d) -> p h d", d=DH))

            # ---- V projection: v[t, c] natural layout, packed as [V_h | 1] per head
            for tt in range(NTT):
                v_ps = ps_mm.tile([128, 1024], f32, tag="mm")
                for dt in range(NDT):
                    nc.tensor.matmul(v_ps[:, 0:CC], xT[:, dt, tt * 128:(tt + 1) * 128],
                                     wv[:, dt], start=(dt == 0), stop=(dt == NDT - 1))
                with nc.allow_low_precision("bf16 v with bias"):
                    nc.vector.tensor_tensor(vp[:, tt, :, 0:DH],
                                            v_ps[:, 0:CC].rearrange("p (h d) -> p h d", d=DH),
                                            bvb[:], add)

            def qk_group(ct, w_sb, dst, sec, tb):
                p_ps = ps_qk.tile([128, 512], f32, tag="qk")
                for dt in range(NDT):
                    nc.tensor.matmul(p_ps[:], w_sb[:, dt, ct * 128:(ct + 1) * 128],
                                     xT[:, dt, tb * 512:(tb + 1) * 512],
                                     start=(dt == 0), stop=(dt == NDT - 1))
                with nc.allow_low_precision("bf16 q/k with bias"):
                    nc.vector.tensor_scalar_add(dst[:, ct, tb * 512:(tb + 1) * 512],
                                                p_ps[:], bpc[:, sec, ct:ct + 1])

            def qk_groups(ct):
                for w_sb, dst, sec in ((wq, qt, 0), (wk, kt, 1)):
                    for tb in range(NTB):
                        yield lambda w_sb=w_sb, dst=dst, sec=sec, tb=tb: qk_group(ct, w_sb, dst, sec, tb)

            def outproj_tt(tt):
                for ch in range(2):
                    o_ps = ps_qk.tile([128, 512], f32, tag="qk")
                    for p4 in range(NCT):
                        nc.tensor.matmul(o_ps[:], yt[:, p4, tt * 128:(tt + 1) * 128],
                                         wo[:, p4, ch * 512:(ch + 1) * 512],
                                         start=(p4 == 0), stop=(p4 == NCT - 1))
                    o_sb = op_.tile([128, 512], f32, tag="osb")
                    nc.scalar.copy(o_sb[:], o_ps[:])
                    nc.sync.dma_start(out_d.ap()[tt * 128:(tt + 1) * 128, ch * 512:(ch + 1) * 512], o_sb[:])

            def attention_block(ct, qb):
                """S -> exp -> AV software-pipelined one k-tile deep."""
                nkt = 4 * (qb + 1)
                yA = ps_y.tile([128, 512], f32, tag="y")
                yB = ps_y.tile([128, 512], f32, tag="y")
                ys = (yA, yB)
                pend = None
                for ki in range(nkt):
                    j = ki - 4 * qb
                    n0 = 128 * j if j >= 0 else 0
                    s_ps = ps_mm.tile([128, 1024], f32, tag="mm")
                    ks = slice(ki * 128, (ki + 1) * 128)
                    qs = slice(qb * 512 + n0, (qb + 1) * 512)
                    nc.tensor.matmul(s_ps[:, n0:512], kt[0:64, ct, ks], qt[0:64, ct, qs],
                                     start=True, stop=True)
                    nc.tensor.matmul(s_ps[:, 512 + n0:1024], kt[64:128, ct, ks], qt[64:128, ct, qs],
                                     start=True, stop=True)
                    pt = wp.tile([128, 2, 512], bf16, tag="pt")
                    s_v = s_ps[:].rearrange("p (c q) -> p c q", q=512)
                    nc.scalar.activation(pt[:, :, n0:512], s_v[:, :, n0:512], Exp, scale=0.125)
                    if j >= 0:
                        nc.gpsimd.affine_select(pt[:, :, n0:n0 + 128], pt[:, :, n0:n0 + 128],
                                                pattern=[[0, 2], [1, 128]],
                                                compare_op=mybir.AluOpType.is_ge,
                                                fill=0.0, base=0, channel_multiplier=-1)
                    if pend is not None:
                        pki, ppt, pn0 = pend
                        for h01 in range(2):
                            nc.tensor.matmul(ys[h01][:, pn0:512], vp[:, pki, 2 * ct + h01, :],
                                             ppt[:, h01, pn0:512],
                                             start=(pki == 0), stop=(pki == nkt - 1))
                    pend = (ki, pt, n0)
                pki, ppt, pn0 = pend
                for h01 in range(2):
                    nc.tensor.matmul(ys[h01][:, pn0:512], vp[:, pki, 2 * ct + h01, :],
                                     ppt[:, h01, pn0:512],
                                     start=(pki == 0), stop=(pki == nkt - 1))
                for h01 in range(2):
                    y_ps = ys[h01]
                    lnz = sp.tile([64, 512], f32, tag="lnz")
                    nc.scalar.activation(lnz[:], y_ps[64:128, :], Ln)
                    rcp = sp.tile([64, 512], bf16, tag="rcp")
                    nc.scalar.activation(rcp[:], lnz[:], Exp, scale=-1.0)
                    with nc.allow_low_precision("bf16 normalized y"):
                        nc.vector.tensor_tensor(yt[64 * h01:64 * (h01 + 1), ct, qb * 512:(qb + 1) * 512],
                                                y_ps[0:64, :], rcp[:], mult)

            # ---- interleaved schedule
            for g in qk_groups(0):
                g()
            for ct in range(NCT):
                filler = list(qk_groups(ct + 1)) if ct + 1 < NCT else None
                for qb in range(NQB):
                    attention_block(ct, qb)
                    if filler is not None:
                        for g in filler[2 * qb:2 * qb + 2]:
                            g()
                    else:
                        for tt in range(4 * qb, 4 * qb + 4):
                            outproj_tt(tt)

    nc.compile()
    return nc


def _get_nc():
    if 'nc' not in _CACHE:
        _CACHE['nc'] = _build()
    return _CACHE['nc']


def _prep_in_maps(x, Wqkv, bqkv, Wout):
    bf = ml_dtypes.bfloat16
    in_maps = []
    per_hg = {}
    for hg in range(2):
        c0 = CC * hg
        wq = np.ascontiguousarray(Wqkv[:, c0:c0 + CC]).astype(bf)
        wk = np.ascontiguousarray(Wqkv[:, DM + c0:DM + c0 + CC]).astype(bf)
        wv = np.ascontiguousarray(Wqkv[:, 2 * DM + c0:2 * DM + c0 + CC]).astype(bf)
        wo = np.ascontiguousarray(Wout[c0:c0 + CC, :]).astype(bf)
        bq = bqkv[c0:c0 + CC]
        bk = bqkv[DM + c0:DM + c0 + CC]
        bv = bqkv[2 * DM + c0:2 * DM + c0 + CC]
        # [128, 2, NCT]: element [p, s, ct] = bias_s[ct*128 + p]
        bpc = np.stack([bq.reshape(NCT, 128).T, bk.reshape(NCT, 128).T], axis=1).astype(np.float32)
        brow = bv.reshape(1, CC).astype(bf)
        per_hg[hg] = dict(wq=wq, wk=wk, wv=wv, wo=wo, bpc=np.ascontiguousarray(bpc), brow=brow)
    for c in range(8):
        b, hg = c // 2, c % 2
        xT = np.ascontiguousarray(x[b].T).astype(bf)
        in_maps.append({"xT": xT, **per_hg[hg]})
    return in_maps


def kernel(x, Wqkv, bqkv, Wout, bout):
    from concourse.bass_utils import run_bass_kernel_spmd
    nc = _get_nc()
    x = np.asarray(x, dtype=np.float32)
    Wqkv = np.asarray(Wqkv, dtype=np.float32)
    bqkv = np.asarray(bqkv, dtype=np.float32)
    Wout = np.asarray(Wout, dtype=np.float32)
    bout = np.asarray(bout, dtype=np.float32)
    in_maps = _prep_in_maps(x, Wqkv, bqkv, Wout)
    res = run_bass_kernel_spmd(nc, in_maps, list(range(8))).results
    out = np.empty((B, T, DM), dtype=np.float32)
    for b in range(B):
        out[b] = res[2 * b]["out"] + res[2 * b + 1]["out"]
    out += bout[None, None, :]
    return out
